# revision 32
# baseline (speedup 1.0000x reference)
"""GCN 2-layer encoder on 8 Trainium2 NeuronCores (Bass/Tile).

Strategy (graph/data parallel, per sharding hint):
 - Nodes sharded by contiguous range across 8 cores (dst side).
 - h1 = x @ W1 computed on each core's shard, AllGather -> full h1 table in HBM.
 - Per-core aggregation over its dst shard: edges sorted (src-bucket major,
   dst-tile minor), gathered from the h1 table via dma_gather (int16 indices
   force 4 source buckets of 25k rows), weighted one-hot matrices built on DVE
   (iota == dstlocal) * norm, contracted on the TensorEngine into PSUM, and
   accumulated per dst tile in SBUF.
 - relu(+b1) fused on ScalarE, projection by W2 on TensorE, AllGather of the
   projected table, second identical aggregation pass, +b2, write out shard.
"""
import numpy as np

NCORES = 8
P = 128
BUCKET = 25000
TB = 7  # dst tiles covered by one merged dma_gather

_CACHE = {}


# ---------------------------------------------------------------- preprocessing
def _balance_perm(deg, n_nodes, shard, ntiles):
    """Within-shard node relabeling: LPT-pack nodes into 128-node tiles so
    every tile has near-equal total in-degree (equalizes chunk counts).
    Returns newpos[node] -> permuted position."""
    import heapq

    newpos = np.empty(n_nodes, dtype=np.int64)
    for c in range(n_nodes // shard):
        lo = c * shard
        nodes = np.arange(lo, lo + shard)
        order = nodes[np.argsort(-deg[lo:lo + shard], kind="stable")]
        sizes = np.full(ntiles, P, dtype=np.int64)
        sizes[ntiles - 1] = shard - (ntiles - 1) * P
        heap = [(0.0, t) for t in range(ntiles)]
        heapq.heapify(heap)
        fill = np.zeros(ntiles, dtype=np.int64)
        for v in order:
            while True:
                s, t = heapq.heappop(heap)
                if fill[t] < sizes[t]:
                    break
            newpos[v] = lo + t * P + fill[t]
            fill[t] += 1
            if fill[t] < sizes[t]:
                heapq.heappush(heap, (s + deg[v], t))
    return newpos


def _prep(edge_index, n_nodes, n_cores=NCORES):
    src = edge_index[0].astype(np.int64)
    dst = edge_index[1].astype(np.int64)
    loops = np.arange(n_nodes, dtype=np.int64)
    src = np.concatenate([src, loops])
    dst = np.concatenate([dst, loops])
    deg = np.bincount(dst, minlength=n_nodes).astype(np.float32)
    dinv = np.where(deg > 0, 1.0 / np.sqrt(deg), 0.0).astype(np.float32)
    norm = (dinv[src] * dinv[dst]).astype(np.float32)

    shard = n_nodes // n_cores
    assert shard * n_cores == n_nodes
    ntiles = (shard + P - 1) // P
    nbkt = (n_nodes + BUCKET - 1) // BUCKET

    newpos = _balance_perm(deg, n_nodes, shard, ntiles)
    # self-edges (incl. the added loops) go through a dedicated per-tile
    # path reading the core-local h1i/gi rows; keep only proper edges here
    selfm = src == dst
    selfnorm = np.zeros(n_nodes, dtype=np.float32)  # by permuted position
    np.add.at(selfnorm, newpos[src[selfm]], norm[selfm])
    src, dst, norm = src[~selfm], dst[~selfm], norm[~selfm]
    src = newpos[src]
    dst = newpos[dst]

    core = dst // shard
    tile_id = (dst % shard) // P
    dstlocal = (dst % shard) % P
    bucket = src // BUCKET
    idx16 = (src % BUCKET).astype(np.int16)

    counts = np.zeros((n_cores, nbkt, ntiles), dtype=np.int64)
    np.add.at(counts, (core, bucket, tile_id), 1)
    K = (counts.max(axis=0) + P - 1) // P  # chunks per (bucket, tile)

    order = np.lexsort((tile_id, bucket, core))
    bucket_s, tile_s = bucket[order], tile_id[order]
    idx_s, dl_s, norm_s = idx16[order], dstlocal[order], norm[order]
    core_s = core[order]

    nchunks = int(K.sum())
    npad = nchunks * P
    idx_pad = np.zeros((n_cores, npad), dtype=np.int16)
    dl_pad = np.zeros((n_cores, npad), dtype=np.float32)
    norm_pad = np.zeros((n_cores, npad), dtype=np.float32)

    # chunk layout: block-major, bucket-minor, tile-innermost. One merged
    # dma_gather covers a whole (bucket, tile-block) run; pads gather row 0
    # (one-hot weight 0 nullifies them) so no per-group count register is
    # needed. Tiles within a block still chain-accumulate in PSUM.
    nblk = (ntiles + TB - 1) // TB
    run_off = np.zeros((nbkt, ntiles), dtype=np.int64)
    off = 0
    merged = []  # (chunk_start, n_chunks, bucket, blk)
    for blk in range(nblk):
        t0, t1 = blk * TB, min((blk + 1) * TB, ntiles)
        for b in range(nbkt):
            g0 = off
            for t in range(t0, t1):
                run_off[b, t] = off
                off += int(K[b, t]) * P
            if off > g0:
                merged.append((g0 // P, (off - g0) // P, b, blk))
    assert off == nchunks * P

    for c in range(n_cores):
        m = core_s == c
        bs, ts = bucket_s[m], tile_s[m]
        key = (ts // TB) * (nbkt * TB) + bs * TB + (ts % TB)
        sort_idx = np.argsort(key, kind="stable")
        kk = key[sort_idx]
        boundary = np.r_[True, kk[1:] != kk[:-1]] if len(kk) else np.zeros(0, bool)
        grp_start = np.flatnonzero(boundary)
        within = np.arange(len(kk)) - np.repeat(
            grp_start, np.diff(np.r_[grp_start, len(kk)])
        )
        ranks = np.empty_like(key)
        ranks[sort_idx] = within
        slot = run_off[bs, ts] + ranks
        idx_pad[c, slot] = idx_s[m]
        dl_pad[c, slot] = dl_s[m]
        norm_pad[c, slot] = norm_s[m]

    kblkmax = max(k for (_s, k, _b, _blk) in merged)
    return dict(
        shard=shard, ntiles=ntiles, nbkt=nbkt, K=K, run_off=run_off,
        merged=merged, nblk=nblk, kblkmax=kblkmax, idx_pad=idx_pad,
        dl_pad=dl_pad, norm_pad=norm_pad, nchunks=nchunks, newpos=newpos,
        selfnorm=selfnorm,
    )


def _pack_idx(idx_pad_c):
    """[npad] int16 -> [128, npad//16] wrapped in 16 partitions, replicated x8."""
    npad = idx_pad_c.shape[0]
    t = idx_pad_c.reshape(npad // 16, 16).T  # [16, cols]
    return np.ascontiguousarray(np.tile(t, (8, 1)))


# ---------------------------------------------------------------- device build
def _build(pp, fin, fh, fo, repeat=1, rep_phases=("A", "AG1", "B", "AG2", "D"),
           nq=1):
    import concourse.bass as bass
    import concourse.bacc as bacc
    import concourse.tile as tile
    import concourse.mybir as mybir
    from concourse.tile_rust import add_dep_helper

    f32 = mybir.dt.float32
    bf16 = mybir.dt.bfloat16
    shard, ntiles, nbkt = pp["shard"], pp["ntiles"], pp["nbkt"]
    nchunks = pp["nchunks"]
    K, run_off = pp["K"], pp["run_off"]
    merged, nblk, kblkmax = pp["merged"], pp["nblk"], pp["kblkmax"]
    kin = fin // P  # input-channel chunks (2)

    import os
    scratch = int(os.environ.get("DMA_SCRATCH", "16384"))
    nc = bacc.Bacc("TRN2", target_bir_lowering=False, debug=False,
                   num_devices=NCORES, num_swdge_queues=nq,
                   dynamic_dma_scratch_size=scratch)
    xT = nc.dram_tensor("xT", [fin, shard], f32, kind="ExternalInput")
    W1 = nc.dram_tensor("W1", [fin, fh], f32, kind="ExternalInput")
    W2b = nc.dram_tensor("W2b", [fh, fo], bf16, kind="ExternalInput")
    b1c = nc.dram_tensor("b1c", [fh, 1], f32, kind="ExternalInput")
    b2r = nc.dram_tensor("b2r", [P, fo], f32, kind="ExternalInput")
    iota_d = nc.dram_tensor("iota", [P, P], bf16, kind="ExternalInput")
    idx_d = nc.dram_tensor("idxt", [P, nchunks * 8], mybir.dt.int16,
                           kind="ExternalInput")
    meta_d = nc.dram_tensor("meta", [P, 2, nchunks + ntiles], f32,
                            kind="ExternalInput")
    outp = nc.dram_tensor("outp", [shard, fo], f32, kind="ExternalOutput")

    xT_v = xT.ap().rearrange("(a p) n -> p a n", p=P)
    W1_v = W1.ap().rearrange("(a p) c -> p a c", p=P)

    with tile.TileContext(nc) as tc:
        with (
            tc.tile_pool(name="const", bufs=1) as constp,
            tc.tile_pool(name="dram", bufs=1, space="DRAM") as dram,
        ):
            # constants
            w1_sb = constp.tile([P, kin, fh], f32)
            nc.sync.dma_start(out=w1_sb[:], in_=W1_v[:])
            w2_sb = constp.tile([P, fo], bf16)
            nc.sync.dma_start(out=w2_sb[:], in_=W2b.ap()[:])
            b1_sb = constp.tile([P, 1], f32)
            nc.sync.dma_start(out=b1_sb[:], in_=b1c.ap()[:])
            b2_sb = constp.tile([P, fo], f32)
            nc.sync.dma_start(out=b2_sb[:], in_=b2r.ap()[:])
            iota_sb = constp.tile([P, P], bf16)
            nc.sync.dma_start(out=iota_sb[:], in_=iota_d.ap()[:])
            idx_all = constp.tile([P, nchunks * 8], mybir.dt.int16)
            nc.sync.dma_start(out=idx_all[:], in_=idx_d.ap()[:])
            meta_all = constp.tile([P, 2, nchunks + ntiles], f32)
            nc.sync.dma_start(out=meta_all[:], in_=meta_d.ap()[:])

            h1i = dram.tile([shard, fh], bf16, name="h1i")
            gi = dram.tile([shard, P], bf16, name="gi")  # fo cols + pad
            cur = {}  # current AG output tiles (fresh per AG execution)
            last_out_dma = [None]  # last outp write of previous rep

            def phase_a():
                AB = 8  # tiles per load/store batch
                with (
                    tc.tile_pool(name="pa_sb", bufs=3) as pa_sb,
                    tc.tile_pool(name="pa_ps", bufs=4, space="PSUM") as pa_ps,
                ):
                    for t0 in range(0, ntiles, AB):
                        lo = t0 * P
                        nw = min(AB * P, shard - lo)
                        nt = (nw + P - 1) // P
                        xt = pa_sb.tile([P, kin, AB * P], f32, tag="xt")
                        xdma = nc.sync.dma_start(out=xt[:, :, :nw],
                                                 in_=xT_v[:, :, lo:lo + nw])
                        if t0 == 0 and last_out_dma[0] is not None:
                            # serialize repeats for honest per-rep timing
                            add_dep_helper(xdma.ins, last_out_dma[0].ins,
                                           sync=True, reason="rep chain")
                        hsb = pa_sb.tile([P, AB, fh], bf16, tag="hsb")
                        for s in range(nt):
                            sw = min(P, nw - s * P)
                            ps = pa_ps.tile([P, fh], f32, tag="ps")
                            for a in range(kin):
                                nc.tensor.matmul(
                                    out=ps[:sw, :],
                                    lhsT=xt[:, a, s * P:s * P + sw],
                                    rhs=w1_sb[:, a, :], start=(a == 0),
                                    stop=(a == kin - 1))
                            nc.vector.tensor_copy(out=hsb[:sw, s, :],
                                                  in_=ps[:sw, :])
                        if nw == nt * P:
                            h1i_v = h1i[lo:lo + nw, :].rearrange(
                                "(a p) c -> p a c", p=P)
                            nc.sync.dma_start(out=h1i_v, in_=hsb[:, :nt, :])
                        else:  # ragged tail: per-subtile stores
                            for s in range(nt):
                                sw = min(P, nw - s * P)
                                nc.sync.dma_start(
                                    out=h1i[lo + s * P:lo + s * P + sw, :],
                                    in_=hsb[:sw, s, :])

            _agn = [0]

            def ag(src_t, width, key):
                dst_t = dram.tile([shard * NCORES, width], bf16,
                                  name=f"{key}_{_agn[0]}", addr_space="Shared")
                _agn[0] += 1
                nc.gpsimd.collective_compute(
                    "AllGather", mybir.AluOpType.bypass,
                    replica_groups=[list(range(NCORES))],
                    ins=[src_t.opt()], outs=[dst_t.opt()],
                )
                cur[key] = dst_t

            def agg_pass(table, self_t, F, acc_w, lhs_is_msgs, rhs_cols,
                         epilogue):
                # merged gathers per (bucket, tile-block); tiles chain-
                # accumulate their chunks in PSUM (self chunk first, loaded
                # from the core-local table); epilogue reads PSUM
                with (
                    tc.tile_pool(name=f"gb{F}{lhs_is_msgs}", bufs=1) as gpool,
                    tc.tile_pool(name=f"oh{F}{lhs_is_msgs}", bufs=6) as ohpool,
                    tc.tile_pool(name=f"ps{F}{lhs_is_msgs}", bufs=4,
                                 space="PSUM") as pspool,
                ):
                    gbufs = [
                        gpool.tile([P, kblkmax, F], bf16, tag=f"gb{bi_}",
                                   name=f"gbuf{F}{lhs_is_msgs}_{bi_}")
                        for bi_ in range(2 * nbkt)
                    ]
                    sbufs = []
                    for bi_ in range(3):
                        sz = gpool.tile([P, F], bf16, tag=f"sf{bi_}",
                                        name=f"sbuf{F}{lhs_is_msgs}_{bi_}")
                        nc.vector.memset(sz[:], 0.0)
                        sbufs.append(sz)
                    mg = {(blk, b): (s, k) for (s, k, b, blk) in merged}
                    gmap = {}
                    gctr = 0
                    for blk in range(nblk):
                        for b in range(nbkt):
                            if (blk, b) not in mg:
                                continue
                            s, k = mg[(blk, b)]
                            gb = gbufs[gctr % (2 * nbkt)]
                            nc.gpsimd.dma_gather(
                                out_ap=gb[:, :k, :],
                                in_ap=table.opt()[
                                    b * BUCKET:min((b + 1) * BUCKET,
                                                   shard * NCORES), :],
                                idxs_ap=idx_all[:, s * 8:(s + k) * 8],
                                num_idxs=k * P,
                                num_idxs_reg=k * P,
                                elem_size=F,
                                single_packet=False,
                                queue_num=gctr % nq,
                            )
                            gmap[(blk, b)] = (gb, s)
                            gctr += 1
                        for t in range(blk * TB, min((blk + 1) * TB, ntiles)):
                            lo = t * P
                            nw = min(P, shard - lo)
                            nch = int(K[:, t].sum()) + 1
                            ps = pspool.tile([P, acc_w], f32, tag="ps")
                            # self chunk: contiguous rows of the local table
                            sf = sbufs[t % 3]
                            nc.sync.dma_start(out=sf[:nw, :],
                                              in_=self_t[lo:lo + nw, :])
                            ci = nchunks + t
                            oh = ohpool.tile([P, P], bf16, tag="oh")
                            nc.vector.tensor_scalar(
                                out=oh[:], in0=iota_sb[:],
                                scalar1=meta_all[:, 0, ci:ci + 1],
                                scalar2=meta_all[:, 1, ci:ci + 1],
                                op0=mybir.AluOpType.is_equal,
                                op1=mybir.AluOpType.mult)
                            if lhs_is_msgs:
                                nc.tensor.matmul(out=ps[:], lhsT=sf[:],
                                                 rhs=oh[:], start=True,
                                                 stop=(nch == 1))
                            else:
                                nc.tensor.matmul(out=ps[:], lhsT=oh[:],
                                                 rhs=sf[:, :rhs_cols],
                                                 start=True, stop=(nch == 1))
                            jall = 1
                            for b in range(nbkt):
                                k = int(K[b, t])
                                if k == 0:
                                    continue
                                gb, s0 = gmap[(blk, b)]
                                coff = run_off[b, t] // P - s0
                                for j in range(k):
                                    ci = run_off[b, t] // P + j
                                    oh = ohpool.tile([P, P], bf16, tag="oh")
                                    nc.vector.tensor_scalar(
                                        out=oh[:], in0=iota_sb[:],
                                        scalar1=meta_all[:, 0, ci:ci + 1],
                                        scalar2=meta_all[:, 1, ci:ci + 1],
                                        op0=mybir.AluOpType.is_equal,
                                        op1=mybir.AluOpType.mult)
                                    if lhs_is_msgs:
                                        nc.tensor.matmul(
                                            out=ps[:], lhsT=gb[:, coff + j, :],
                                            rhs=oh[:], start=False,
                                            stop=(jall == nch - 1))
                                    else:
                                        nc.tensor.matmul(
                                            out=ps[:], lhsT=oh[:],
                                            rhs=gb[:, coff + j, :rhs_cols],
                                            start=False,
                                            stop=(jall == nch - 1))
                                    jall += 1
                            epilogue(t, ps)

            def phase_b():
                with (
                    tc.tile_pool(name="ep1", bufs=3) as ep1,
                    tc.tile_pool(name="ep1ps", bufs=2, space="PSUM") as ep1ps,
                ):
                    def epi1(t, ps):
                        lo = t * P
                        nw = min(P, shard - lo)
                        z = ep1.tile([P, P], bf16, tag="z")
                        nc.scalar.activation(
                            out=z[:], in_=ps[:],
                            func=mybir.ActivationFunctionType.Relu,
                            bias=b1_sb[:, 0:1])
                        ps2 = ep1ps.tile([P, fo], f32, tag="ps2")
                        nc.tensor.matmul(out=ps2[:], lhsT=z[:], rhs=w2_sb[:],
                                         start=True, stop=True)
                        h2 = ep1.tile([P, P], bf16, tag="h2")
                        nc.vector.memset(h2[:, fo:], 0.0)
                        nc.vector.tensor_copy(out=h2[:, :fo], in_=ps2[:])
                        nc.sync.dma_start(out=gi[lo:lo + nw, :],
                                          in_=h2[:nw, :])

                    agg_pass(cur["h1f"], h1i, fh, P, lhs_is_msgs=True,
                             rhs_cols=None, epilogue=epi1)

            def phase_d():
                with tc.tile_pool(name="ep2", bufs=3) as ep2:
                    def epi2(t, ps):
                        lo = t * P
                        nw = min(P, shard - lo)
                        o = ep2.tile([P, fo], f32, tag="o")
                        nc.vector.tensor_tensor(out=o[:], in0=ps[:],
                                                in1=b2_sb[:],
                                                op=mybir.AluOpType.add)
                        last_out_dma[0] = nc.sync.dma_start(
                            out=outp.ap()[lo:lo + nw, :], in_=o[:nw, :])

                    agg_pass(cur["gf"], gi, P, fo, lhs_is_msgs=False,
                             rhs_cols=fo, epilogue=epi2)

            phase_fns = {"A": phase_a,
                         "AG1": lambda: ag(h1i, fh, "h1f"),
                         "B": phase_b,
                         "AG2": lambda: ag(gi, P, "gf"),
                         "D": phase_d}
            for _rep in range(repeat):
                for ph in ("A", "AG1", "B", "AG2", "D"):
                    if _rep == 0 or ph in rep_phases:
                        phase_fns[ph]()

    nc.compile()
    return nc


# ---------------------------------------------------------------- entry point
def _make_in_maps(pp, x, W1, b1, W2, b2):
    import ml_dtypes
    bf16 = ml_dtypes.bfloat16

    fh = W1.shape[1]
    fo = W2.shape[1]
    shard = pp["shard"]
    iota = np.tile(np.arange(P, dtype=np.float32)[None, :], (P, 1))
    b1c = b1.reshape(fh, 1)
    b2r = np.tile(b2[None, :], (P, 1))

    xp = np.empty_like(x)
    xp[pp["newpos"]] = x
    in_maps = []
    for c in range(NCORES):
        xT = np.ascontiguousarray(xp[c * shard:(c + 1) * shard, :].T)
        # meta[p, 0, ci] = dstlocal of edge ci*128+p ; [p, 1, ci] = norm.
        # Columns nchunks..nchunks+ntiles are the per-tile self chunks
        # (diagonal: dl=p, norm=selfnorm of the tile's nodes).
        dl = pp["dl_pad"][c].reshape(-1, P).T  # [128, nchunks]
        nm = pp["norm_pad"][c].reshape(-1, P).T
        ntiles = pp["ntiles"]
        snT = np.zeros((ntiles, P), dtype=np.float32)
        snT.reshape(-1)[:shard] = pp["selfnorm"][c * shard:(c + 1) * shard]
        sn = snT.T
        dl_self = np.tile(np.arange(P, dtype=np.float32)[:, None],
                          (1, ntiles))
        dl = np.concatenate([dl, dl_self], axis=1)
        nm = np.concatenate([nm, sn], axis=1)
        meta = np.ascontiguousarray(np.stack([dl, nm], axis=1))
        in_maps.append({
            "xT": xT, "W1": W1, "W2b": W2.astype(bf16), "b1c": b1c,
            "b2r": b2r, "iota": iota.astype(bf16),
            "idxt": _pack_idx(pp["idx_pad"][c]), "meta": meta,
        })
    return in_maps


def kernel(x, edge_index, W1, b1, W2, b2, _want_results=False, _trace=False):
    import concourse.bass_utils as bass_utils

    x = np.ascontiguousarray(np.asarray(x, dtype=np.float32))
    ei = np.asarray(edge_index).astype(np.int64)
    W1 = np.asarray(W1, dtype=np.float32)
    b1 = np.asarray(b1, dtype=np.float32)
    W2 = np.asarray(W2, dtype=np.float32)
    b2 = np.asarray(b2, dtype=np.float32)
    n, fin = x.shape
    fh = W1.shape[1]
    fo = W2.shape[1]

    key = ("v3", n, fin, fh, fo, int(ei[0, :8].sum()), int(ei[1, :8].sum()),
           ei.shape[1])
    if key in _CACHE:
        nc, pp = _CACHE[key]
    else:
        pp = _prep(ei, n)
        nc = _build(pp, fin, fh, fo)
        _CACHE[key] = (nc, pp)

    in_maps = _make_in_maps(pp, x, W1, b1, W2, b2)

    res = bass_utils.run_bass_kernel_spmd(
        nc, in_maps, core_ids=list(range(NCORES)), trace=_trace)
    outp = np.concatenate([res.results[c]["outp"] for c in range(NCORES)],
                          axis=0)
    out = outp[pp["newpos"]]  # undo the load-balancing permutation
    if _want_results:
        return out, res
    return out



# revision 41
# speedup vs baseline: 4808.2302x; 4808.2302x over previous
"""GCN 2-layer encoder on 8 Trainium2 NeuronCores (Bass/Tile).

Strategy (graph/data parallel, per sharding hint): nodes are sharded by
contiguous range across the 8 cores on the destination side, the small
W/b are replicated, and gathered source features cross cores through two
bf16 AllGather'd feature tables.

 - Node relabeling (host): within each shard, nodes are LPT-packed into
   128-node dst tiles with near-equal total in-degree, which equalizes the
   per-(bucket, tile) edge-chunk counts across cores (the SPMD instruction
   stream must cover the max). The output is un-permuted on the host.
 - Phase A: h1 = x @ W1 per shard (fp32 matmul, x loaded via ScalarE-issued
   HWDGE in pipelined chunks), cast to bf16, AllGather -> full h1 table.
 - Aggregation passes (B over h1, D over the gi table): per dst tile, edge
   chunks of 128 are contracted on TensorE as msgs^T @ onehot (layer 1) /
   onehot^T @ msgs (layer 2), chain-accumulated in one PSUM tile across all
   4 src buckets. One merged dma_gather per (bucket, 7-tile block) fetches
   the bf16 message rows (int16 indices force 25k-row buckets; pads gather
   row 0 and carry zero one-hot weight). One-hot matrices are built on DVE
   in bf16 4x mode as (iota == dstlocal) * norm. Self-loops bypass the
   gather entirely: each tile's own h1i/gi rows are DMA'd contiguously and
   contracted with a diagonal one-hot, which also debiases bucket counts.
 - Epilogues run from PSUM: ScalarE fuses relu(+b1) and the bf16 cast;
   the W2 projection (zero-padded to 128 cols) goes back through TensorE;
   +b2 is folded in as one extra rank-1 matmul on the PSUM chain; ScalarE
   copies results out so the DVE stays dedicated to one-hot builds.
 - gi (layer-2 table) is bf16 padded to 128 cols so gathered rows meet the
   256B dma_gather granularity; AllGather #2 ships it, phase D repeats the
   aggregation and writes the fp32 output shard.
"""
import numpy as np

NCORES = 8
P = 128
BUCKET = 25000
TB = 7  # dst tiles covered by one merged dma_gather

_CACHE = {}


# ---------------------------------------------------------------- preprocessing
def _balance_perm(deg, n_nodes, shard, ntiles):
    """Within-shard node relabeling: LPT-pack nodes into 128-node tiles so
    every tile has near-equal total in-degree (equalizes chunk counts).
    Returns newpos[node] -> permuted position."""
    import heapq

    newpos = np.empty(n_nodes, dtype=np.int64)
    for c in range(n_nodes // shard):
        lo = c * shard
        nodes = np.arange(lo, lo + shard)
        order = nodes[np.argsort(-deg[lo:lo + shard], kind="stable")]
        sizes = np.full(ntiles, P, dtype=np.int64)
        sizes[ntiles - 1] = shard - (ntiles - 1) * P
        heap = [(0.0, t) for t in range(ntiles)]
        heapq.heapify(heap)
        fill = np.zeros(ntiles, dtype=np.int64)
        for v in order:
            while True:
                s, t = heapq.heappop(heap)
                if fill[t] < sizes[t]:
                    break
            newpos[v] = lo + t * P + fill[t]
            fill[t] += 1
            if fill[t] < sizes[t]:
                heapq.heappush(heap, (s + deg[v], t))
    return newpos


def _prep(edge_index, n_nodes, n_cores=NCORES):
    src = edge_index[0].astype(np.int64)
    dst = edge_index[1].astype(np.int64)
    loops = np.arange(n_nodes, dtype=np.int64)
    src = np.concatenate([src, loops])
    dst = np.concatenate([dst, loops])
    deg = np.bincount(dst, minlength=n_nodes).astype(np.float32)
    dinv = np.where(deg > 0, 1.0 / np.sqrt(deg), 0.0).astype(np.float32)
    norm = (dinv[src] * dinv[dst]).astype(np.float32)

    shard = n_nodes // n_cores
    assert shard * n_cores == n_nodes
    ntiles = (shard + P - 1) // P
    nbkt = (n_nodes + BUCKET - 1) // BUCKET

    newpos = _balance_perm(deg, n_nodes, shard, ntiles)
    # self-edges (incl. the added loops) go through a dedicated per-tile
    # path reading the core-local h1i/gi rows; keep only proper edges here
    selfm = src == dst
    selfnorm = np.zeros(n_nodes, dtype=np.float32)  # by permuted position
    np.add.at(selfnorm, newpos[src[selfm]], norm[selfm])
    src, dst, norm = src[~selfm], dst[~selfm], norm[~selfm]
    src = newpos[src]
    dst = newpos[dst]

    core = dst // shard
    tile_id = (dst % shard) // P
    dstlocal = (dst % shard) % P
    bucket = src // BUCKET
    idx16 = (src % BUCKET).astype(np.int16)

    counts = np.zeros((n_cores, nbkt, ntiles), dtype=np.int64)
    np.add.at(counts, (core, bucket, tile_id), 1)
    K = (counts.max(axis=0) + P - 1) // P  # chunks per (bucket, tile)

    order = np.lexsort((tile_id, bucket, core))
    bucket_s, tile_s = bucket[order], tile_id[order]
    idx_s, dl_s, norm_s = idx16[order], dstlocal[order], norm[order]
    core_s = core[order]

    nchunks = int(K.sum())
    npad = nchunks * P
    idx_pad = np.zeros((n_cores, npad), dtype=np.int16)
    dl_pad = np.zeros((n_cores, npad), dtype=np.float32)
    norm_pad = np.zeros((n_cores, npad), dtype=np.float32)

    # chunk layout: block-major, bucket-minor, tile-innermost. One merged
    # dma_gather covers a whole (bucket, tile-block) run; pads gather row 0
    # (one-hot weight 0 nullifies them) so no per-group count register is
    # needed. Tiles within a block still chain-accumulate in PSUM.
    nblk = (ntiles + TB - 1) // TB
    run_off = np.zeros((nbkt, ntiles), dtype=np.int64)
    off = 0
    merged = []  # (chunk_start, n_chunks, bucket, blk)
    for blk in range(nblk):
        t0, t1 = blk * TB, min((blk + 1) * TB, ntiles)
        for b in range(nbkt):
            g0 = off
            for t in range(t0, t1):
                run_off[b, t] = off
                off += int(K[b, t]) * P
            if off > g0:
                merged.append((g0 // P, (off - g0) // P, b, blk))
    assert off == nchunks * P

    for c in range(n_cores):
        m = core_s == c
        bs, ts = bucket_s[m], tile_s[m]
        key = (ts // TB) * (nbkt * TB) + bs * TB + (ts % TB)
        sort_idx = np.argsort(key, kind="stable")
        kk = key[sort_idx]
        boundary = np.r_[True, kk[1:] != kk[:-1]] if len(kk) else np.zeros(0, bool)
        grp_start = np.flatnonzero(boundary)
        within = np.arange(len(kk)) - np.repeat(
            grp_start, np.diff(np.r_[grp_start, len(kk)])
        )
        ranks = np.empty_like(key)
        ranks[sort_idx] = within
        slot = run_off[bs, ts] + ranks
        idx_pad[c, slot] = idx_s[m]
        dl_pad[c, slot] = dl_s[m]
        norm_pad[c, slot] = norm_s[m]

    kblkmax = max(k for (_s, k, _b, _blk) in merged)
    return dict(
        shard=shard, ntiles=ntiles, nbkt=nbkt, K=K, run_off=run_off,
        merged=merged, nblk=nblk, kblkmax=kblkmax, idx_pad=idx_pad,
        dl_pad=dl_pad, norm_pad=norm_pad, nchunks=nchunks, newpos=newpos,
        selfnorm=selfnorm,
    )


def _pack_idx(idx_pad_c):
    """[npad] int16 -> [128, npad//16] wrapped in 16 partitions, replicated x8."""
    npad = idx_pad_c.shape[0]
    t = idx_pad_c.reshape(npad // 16, 16).T  # [16, cols]
    return np.ascontiguousarray(np.tile(t, (8, 1)))


# ---------------------------------------------------------------- device build
def _build(pp, fin, fh, fo, repeat=1, rep_phases=("A", "AG1", "B", "AG2", "D"),
           nq=1):
    import concourse.bass as bass
    import concourse.bacc as bacc
    import concourse.tile as tile
    import concourse.mybir as mybir
    from concourse.tile_rust import add_dep_helper

    f32 = mybir.dt.float32
    bf16 = mybir.dt.bfloat16
    shard, ntiles, nbkt = pp["shard"], pp["ntiles"], pp["nbkt"]
    nchunks = pp["nchunks"]
    K, run_off = pp["K"], pp["run_off"]
    merged, nblk, kblkmax = pp["merged"], pp["nblk"], pp["kblkmax"]
    kin = fin // P  # input-channel chunks (2)

    import os
    scratch = int(os.environ.get("DMA_SCRATCH", "16384"))
    nc = bacc.Bacc("TRN2", target_bir_lowering=False, debug=False,
                   num_devices=NCORES, num_swdge_queues=nq,
                   dynamic_dma_scratch_size=scratch)
    xT = nc.dram_tensor("xT", [fin, shard], f32, kind="ExternalInput")
    W1 = nc.dram_tensor("W1", [fin, fh], f32, kind="ExternalInput")
    W2b = nc.dram_tensor("W2b", [fh, P], bf16, kind="ExternalInput")
    b1c = nc.dram_tensor("b1c", [fh, 1], f32, kind="ExternalInput")
    b2r = nc.dram_tensor("b2r", [P, fo], bf16, kind="ExternalInput")
    bones_d = nc.dram_tensor("bones", [P, P], bf16, kind="ExternalInput")
    iota_d = nc.dram_tensor("iota", [P, P], bf16, kind="ExternalInput")
    idx_d = nc.dram_tensor("idxt", [P, nchunks * 8], mybir.dt.int16,
                           kind="ExternalInput")
    meta_d = nc.dram_tensor("meta", [P, 2, nchunks + ntiles], f32,
                            kind="ExternalInput")
    outp = nc.dram_tensor("outp", [shard, fo], f32, kind="ExternalOutput")

    xT_v = xT.ap().rearrange("(a p) n -> p a n", p=P)
    W1_v = W1.ap().rearrange("(a p) c -> p a c", p=P)

    with tile.TileContext(nc) as tc:
        with (
            tc.tile_pool(name="const", bufs=1) as constp,
            tc.tile_pool(name="dram", bufs=1, space="DRAM") as dram,
        ):
            # constants
            w1_sb = constp.tile([P, kin, fh], f32)
            nc.sync.dma_start(out=w1_sb[:], in_=W1_v[:])
            w2_sb = constp.tile([P, P], bf16)  # fo cols + zero pad
            nc.sync.dma_start(out=w2_sb[:], in_=W2b.ap()[:])
            b1_sb = constp.tile([P, 1], f32)
            nc.sync.dma_start(out=b1_sb[:], in_=b1c.ap()[:])
            b2_sb = constp.tile([P, fo], bf16)
            nc.sync.dma_start(out=b2_sb[:], in_=b2r.ap()[:])
            bones_sb = constp.tile([P, P], bf16)
            nc.sync.dma_start(out=bones_sb[:], in_=bones_d.ap()[:])
            iota_sb = constp.tile([P, P], bf16)
            nc.sync.dma_start(out=iota_sb[:], in_=iota_d.ap()[:])
            idx_all = constp.tile([P, nchunks * 8], mybir.dt.int16)
            nc.sync.dma_start(out=idx_all[:], in_=idx_d.ap()[:])
            meta_all = constp.tile([P, 2, nchunks + ntiles], f32)
            nc.sync.dma_start(out=meta_all[:], in_=meta_d.ap()[:])

            h1i = dram.tile([shard, fh], bf16, name="h1i")
            gi = dram.tile([shard, P], bf16, name="gi")  # fo cols + pad
            cur = {}  # current AG output tiles (fresh per AG execution)
            last_out_dma = [None]  # last outp write of previous rep

            def phase_a():
                AB = 8  # tiles per store batch
                XB = 25  # tiles per load chunk (4 pipelined loads)
                with (
                    tc.tile_pool(name="pa_x", bufs=1) as pa_x,
                    tc.tile_pool(name="pa_sb", bufs=3) as pa_sb,
                    tc.tile_pool(name="pa_ps", bufs=4, space="PSUM") as pa_ps,
                ):
                    # x loads issued from the (otherwise idle) ScalarE HWDGE
                    # queue in 4 chunks so PE overlaps the loads
                    xt = pa_x.tile([P, kin, shard], f32, tag="xt")
                    for l0 in range(0, shard, XB * P):
                        lw = min(XB * P, shard - l0)
                        xdma = nc.scalar.dma_start(
                            out=xt[:, :, l0:l0 + lw],
                            in_=xT_v[:, :, l0:l0 + lw])
                        if l0 == 0 and last_out_dma[0] is not None:
                            # serialize repeats for honest per-rep timing
                            add_dep_helper(xdma.ins, last_out_dma[0].ins,
                                           sync=True, reason="rep chain")
                    for t0 in range(0, ntiles, AB):
                        lo = t0 * P
                        nw = min(AB * P, shard - lo)
                        nt = (nw + P - 1) // P
                        hsb = pa_sb.tile([P, AB, fh], bf16, tag="hsb")
                        for s in range(nt):
                            sw = min(P, nw - s * P)
                            ps = pa_ps.tile([P, fh], f32, tag="ps")
                            for a in range(kin):
                                nc.tensor.matmul(
                                    out=ps[:sw, :],
                                    lhsT=xt[:, a, lo + s * P:lo + s * P + sw],
                                    rhs=w1_sb[:, a, :], start=(a == 0),
                                    stop=(a == kin - 1))
                            nc.vector.tensor_copy(out=hsb[:sw, s, :],
                                                  in_=ps[:sw, :])
                        if nw == nt * P:
                            h1i_v = h1i[lo:lo + nw, :].rearrange(
                                "(a p) c -> p a c", p=P)
                            nc.sync.dma_start(out=h1i_v, in_=hsb[:, :nt, :])
                        else:  # ragged tail: per-subtile stores
                            for s in range(nt):
                                sw = min(P, nw - s * P)
                                nc.sync.dma_start(
                                    out=h1i[lo + s * P:lo + s * P + sw, :],
                                    in_=hsb[:sw, s, :])

            _agn = [0]

            def ag(src_t, width, key):
                dst_t = dram.tile([shard * NCORES, width], bf16,
                                  name=f"{key}_{_agn[0]}", addr_space="Shared")
                _agn[0] += 1
                nc.gpsimd.collective_compute(
                    "AllGather", mybir.AluOpType.bypass,
                    replica_groups=[list(range(NCORES))],
                    ins=[src_t.opt()], outs=[dst_t.opt()],
                )
                cur[key] = dst_t

            def agg_pass(table, self_t, F, acc_w, lhs_is_msgs, rhs_cols,
                         epilogue, tail_mm=False):
                # merged gathers per (bucket, tile-block); tiles chain-
                # accumulate their chunks in PSUM (self chunk first, loaded
                # from the core-local table); epilogue reads PSUM
                with (
                    tc.tile_pool(name=f"gb{F}{lhs_is_msgs}", bufs=1) as gpool,
                    tc.tile_pool(name=f"oh{F}{lhs_is_msgs}", bufs=8) as ohpool,
                    tc.tile_pool(name=f"ps{F}{lhs_is_msgs}", bufs=6,
                                 space="PSUM") as pspool,
                ):
                    gbufs = [
                        gpool.tile([P, kblkmax, F], bf16, tag=f"gb{bi_}",
                                   name=f"gbuf{F}{lhs_is_msgs}_{bi_}")
                        for bi_ in range(2 * nbkt)
                    ]
                    sbufs = []
                    for bi_ in range(3):
                        sz = gpool.tile([P, F], bf16, tag=f"sf{bi_}",
                                        name=f"sbuf{F}{lhs_is_msgs}_{bi_}")
                        nc.vector.memset(sz[:], 0.0)
                        sbufs.append(sz)
                    mg = {(blk, b): (s, k) for (s, k, b, blk) in merged}
                    gmap = {}
                    gctr = 0
                    for blk in range(nblk):
                        for b in range(nbkt):
                            if (blk, b) not in mg:
                                continue
                            s, k = mg[(blk, b)]
                            gb = gbufs[gctr % (2 * nbkt)]
                            nc.gpsimd.dma_gather(
                                out_ap=gb[:, :k, :],
                                in_ap=table.opt()[
                                    b * BUCKET:min((b + 1) * BUCKET,
                                                   shard * NCORES), :],
                                idxs_ap=idx_all[:, s * 8:(s + k) * 8],
                                num_idxs=k * P,
                                num_idxs_reg=k * P,
                                elem_size=F,
                                single_packet=False,
                                queue_num=gctr % nq,
                            )
                            gmap[(blk, b)] = (gb, s)
                            gctr += 1
                        for t in range(blk * TB, min((blk + 1) * TB, ntiles)):
                            lo = t * P
                            nw = min(P, shard - lo)
                            nch = int(K[:, t].sum()) + 1 + (1 if tail_mm
                                                             else 0)
                            ps = pspool.tile([P, acc_w], f32, tag="ps")
                            # self chunk: contiguous rows of the local table
                            sf = sbufs[t % 3]
                            nc.sync.dma_start(out=sf[:nw, :],
                                              in_=self_t[lo:lo + nw, :])
                            ci = nchunks + t
                            oh = ohpool.tile([P, P], bf16, tag="oh")
                            nc.vector.tensor_scalar(
                                out=oh[:], in0=iota_sb[:],
                                scalar1=meta_all[:, 0, ci:ci + 1],
                                scalar2=meta_all[:, 1, ci:ci + 1],
                                op0=mybir.AluOpType.is_equal,
                                op1=mybir.AluOpType.mult)
                            if lhs_is_msgs:
                                nc.tensor.matmul(out=ps[:], lhsT=sf[:],
                                                 rhs=oh[:], start=True,
                                                 stop=(nch == 1))
                            else:
                                nc.tensor.matmul(out=ps[:], lhsT=oh[:],
                                                 rhs=sf[:, :rhs_cols],
                                                 start=True, stop=(nch == 1))
                            jall = 1
                            for b in range(nbkt):
                                k = int(K[b, t])
                                if k == 0:
                                    continue
                                gb, s0 = gmap[(blk, b)]
                                coff = run_off[b, t] // P - s0
                                for j in range(k):
                                    ci = run_off[b, t] // P + j
                                    oh = ohpool.tile([P, P], bf16, tag="oh")
                                    nc.vector.tensor_scalar(
                                        out=oh[:], in0=iota_sb[:],
                                        scalar1=meta_all[:, 0, ci:ci + 1],
                                        scalar2=meta_all[:, 1, ci:ci + 1],
                                        op0=mybir.AluOpType.is_equal,
                                        op1=mybir.AluOpType.mult)
                                    if lhs_is_msgs:
                                        nc.tensor.matmul(
                                            out=ps[:], lhsT=gb[:, coff + j, :],
                                            rhs=oh[:], start=False,
                                            stop=(jall == nch - 1))
                                    else:
                                        nc.tensor.matmul(
                                            out=ps[:], lhsT=oh[:],
                                            rhs=gb[:, coff + j, :rhs_cols],
                                            start=False,
                                            stop=(jall == nch - 1))
                                    jall += 1
                            epilogue(t, ps)

            def phase_b():
                with (
                    tc.tile_pool(name="ep1", bufs=3) as ep1,
                    tc.tile_pool(name="ep1ps", bufs=2, space="PSUM") as ep1ps,
                ):
                    def epi1(t, ps):
                        lo = t * P
                        nw = min(P, shard - lo)
                        z = ep1.tile([P, P], bf16, tag="z")
                        nc.scalar.activation(
                            out=z[:], in_=ps[:],
                            func=mybir.ActivationFunctionType.Relu,
                            bias=b1_sb[:, 0:1])
                        ps2 = ep1ps.tile([P, P], f32, tag="ps2")
                        nc.tensor.matmul(out=ps2[:], lhsT=z[:], rhs=w2_sb[:],
                                         start=True, stop=True)
                        h2 = ep1.tile([P, P], bf16, tag="h2")
                        nc.scalar.activation(
                            out=h2[:], in_=ps2[:],
                            func=mybir.ActivationFunctionType.Copy)
                        nc.sync.dma_start(out=gi[lo:lo + nw, :],
                                          in_=h2[:nw, :])

                    agg_pass(cur["h1f"], h1i, fh, P, lhs_is_msgs=True,
                             rhs_cols=None, epilogue=epi1)

            def phase_d():
                with tc.tile_pool(name="ep2", bufs=3) as ep2:
                    def epi2(t, ps):
                        lo = t * P
                        nw = min(P, shard - lo)
                        nc.tensor.matmul(out=ps[:], lhsT=bones_sb[:],
                                         rhs=b2_sb[:], start=False, stop=True)
                        o = ep2.tile([P, fo], f32, tag="o")
                        nc.scalar.activation(
                            out=o[:], in_=ps[:],
                            func=mybir.ActivationFunctionType.Copy)
                        last_out_dma[0] = nc.sync.dma_start(
                            out=outp.ap()[lo:lo + nw, :], in_=o[:nw, :])

                    agg_pass(cur["gf"], gi, P, fo, lhs_is_msgs=False,
                             rhs_cols=fo, epilogue=epi2, tail_mm=True)

            phase_fns = {"A": phase_a,
                         "AG1": lambda: ag(h1i, fh, "h1f"),
                         "B": phase_b,
                         "AG2": lambda: ag(gi, P, "gf"),
                         "D": phase_d}
            for _rep in range(repeat):
                for ph in ("A", "AG1", "B", "AG2", "D"):
                    if _rep == 0 or ph in rep_phases:
                        phase_fns[ph]()

    nc.compile()
    return nc


# ---------------------------------------------------------------- entry point
def _make_in_maps(pp, x, W1, b1, W2, b2):
    import ml_dtypes
    bf16 = ml_dtypes.bfloat16

    fh = W1.shape[1]
    fo = W2.shape[1]
    shard = pp["shard"]
    iota = np.tile(np.arange(P, dtype=np.float32)[None, :], (P, 1))
    b1c = b1.reshape(fh, 1)
    b2r = np.tile(b2[None, :], (P, 1)).astype(bf16)
    W2p = np.zeros((fh, P), dtype=np.float32)
    W2p[:, :fo] = W2
    bones = np.zeros((P, P), dtype=np.float32)
    bones[0, :] = 1.0

    xp = np.empty_like(x)
    xp[pp["newpos"]] = x
    in_maps = []
    for c in range(NCORES):
        xT = np.ascontiguousarray(xp[c * shard:(c + 1) * shard, :].T)
        # meta[p, 0, ci] = dstlocal of edge ci*128+p ; [p, 1, ci] = norm.
        # Columns nchunks..nchunks+ntiles are the per-tile self chunks
        # (diagonal: dl=p, norm=selfnorm of the tile's nodes).
        dl = pp["dl_pad"][c].reshape(-1, P).T  # [128, nchunks]
        nm = pp["norm_pad"][c].reshape(-1, P).T
        ntiles = pp["ntiles"]
        snT = np.zeros((ntiles, P), dtype=np.float32)
        snT.reshape(-1)[:shard] = pp["selfnorm"][c * shard:(c + 1) * shard]
        sn = snT.T
        dl_self = np.tile(np.arange(P, dtype=np.float32)[:, None],
                          (1, ntiles))
        dl = np.concatenate([dl, dl_self], axis=1)
        nm = np.concatenate([nm, sn], axis=1)
        meta = np.ascontiguousarray(np.stack([dl, nm], axis=1))
        in_maps.append({
            "xT": xT, "W1": W1, "W2b": W2p.astype(bf16), "b1c": b1c,
            "b2r": b2r, "bones": bones.astype(bf16),
            "iota": iota.astype(bf16),
            "idxt": _pack_idx(pp["idx_pad"][c]), "meta": meta,
        })
    return in_maps


def kernel(x, edge_index, W1, b1, W2, b2, _want_results=False, _trace=False):
    import concourse.bass_utils as bass_utils

    x = np.ascontiguousarray(np.asarray(x, dtype=np.float32))
    ei = np.asarray(edge_index).astype(np.int64)
    W1 = np.asarray(W1, dtype=np.float32)
    b1 = np.asarray(b1, dtype=np.float32)
    W2 = np.asarray(W2, dtype=np.float32)
    b2 = np.asarray(b2, dtype=np.float32)
    n, fin = x.shape
    fh = W1.shape[1]
    fo = W2.shape[1]

    key = ("v3", n, fin, fh, fo, int(ei[0, :8].sum()), int(ei[1, :8].sum()),
           ei.shape[1])
    if key in _CACHE:
        nc, pp = _CACHE[key]
    else:
        pp = _prep(ei, n)
        nc = _build(pp, fin, fh, fo)
        _CACHE[key] = (nc, pp)

    in_maps = _make_in_maps(pp, x, W1, b1, W2, b2)

    res = bass_utils.run_bass_kernel_spmd(
        nc, in_maps, core_ids=list(range(NCORES)), trace=_trace)
    outp = np.concatenate([res.results[c]["outp"] for c in range(NCORES)],
                          axis=0)
    out = outp[pp["newpos"]]  # undo the load-balancing permutation
    if _want_results:
        return out, res
    return out

